# revision 4
# baseline (speedup 1.0000x reference)
"""Trainium2 Bass kernel for nn_Column1_20298015441326 (topk_masking).

Reference computation (per branch r of RF=512, fully independent):
  pot[r,t,k] = sum_l rec_field[t,0,r,l] * W[r,k,0,l]      (T=32, K=32, L=2048)
  thr = pot * (pot > 20);  spikes = sign(thr)
  kWTA top-4 winner mask per branch (SpykeTorch get_k_winners semantics,
  ties broken by lower feature index), out = spikes * mask, -> (T,1,K,RF).

Sharding: branch axis across 8 cores (64 branches/core), no cross-core comms.

Per-core device layout:
  branches b = g*4 + rs  (g in [0,16) groups, rs in [0,4) col-tiles)
  x dram (16, 128, 2048): [g, p, rs*512 + c*32 + t] = rec_field[t,0,r,c*128+p]
  w dram (16, 128, 2048): [g, p, rs*512 + c*32 + k] = W[r,k,0,c*128+p]
  PE: per (g,rs): pot[k,t] = sum_c wT_c.T @ xT_c  (contraction l on partitions,
      16 chunks of 128, accumulated in PSUM; 4 branches packed in the 128x128
      array via col tile_position)
  pot_all sbuf (128, 512): [rs*32+k, g*32+t]
  Post-processing on DVE in this layout (reductions along free/t), a 32x32
  block transpose to get per-branch feature axis along free for top-4 (Max8),
  stable tie-break via prefix-scan rank among values equal to the 4th max.
  out dram (128, 512) = spikes * mask, host reassembles (T,1,K,RF).
"""

import numpy as np

import concourse.bacc as bacc
import concourse.mybir as mybir
from concourse import bass_utils
from concourse.tile import TileContext

T = 32
K = 32
RF = 512
L = 2048
TH = 20.0
NCORES = 8
G = 16          # branch groups per core
RS = 4          # branches per group (PE col tiles)
CH = 16         # contraction chunks of 128
F32 = mybir.dt.float32
Ax = mybir.AxisListType
Op = mybir.AluOpType

_CACHE = {}


def build():
    """Build + compile the per-core Bass module (SPMD: same program, 8 cores)."""
    nc = bacc.Bacc("TRN2", target_bir_lowering=False, debug=False, num_devices=NCORES)
    x = nc.dram_tensor("x", (G, 128, RS * CH * T), F32, kind="ExternalInput")
    w = nc.dram_tensor("w", (G, 128, RS * CH * K), F32, kind="ExternalInput")
    iota_d = nc.dram_tensor("iota_t", (128, T), F32, kind="ExternalInput")
    out = nc.dram_tensor("out", (128, G * T), F32, kind="ExternalOutput")

    with TileContext(nc) as tc:
        with tc.tile_pool(name="io", bufs=8) as io, \
             tc.tile_pool(name="psp", bufs=6, space="PSUM") as psp, \
             tc.tile_pool(name="wk", bufs=1) as wk:
            iota_sb = wk.tile([128, T], F32)
            nc.gpsimd.dma_start(out=iota_sb[:], in_=iota_d[:, :])
            zeros = wk.tile([128, K], F32)
            nc.vector.memset(zeros[:], 0.0)

            pot = wk.tile([128, G * T], F32)
            gt = wk.tile([128, G * T], F32)
            thr = wk.tile([128, G * T], F32)
            sel = wk.tile([128, G * T], F32)
            sel2 = wk.tile([128, G * T], F32)
            # packed (128, 96): [cnt | pad | vals | pad | rowmax | pad] (16 each)
            packed = wk.tile([128, 96], F32)
            nc.vector.memset(packed[:], 0.0)
            first = wk.tile([128, G], F32)
            has = wk.tile([128, G], F32)

            def stage_a(glo, ghi):
                """fire + per-feature stats for groups [glo, ghi)."""
                gn = ghi - glo
                fs = slice(glo * T, ghi * T)
                p3 = pot[:, fs].rearrange("p (g t) -> p g t", t=T)
                g3 = gt[:, fs].rearrange("p (g t) -> p g t", t=T)
                t3 = thr[:, fs].rearrange("p (g t) -> p g t", t=T)
                s3 = sel[:, fs].rearrange("p (g t) -> p g t", t=T)
                s23 = sel2[:, fs].rearrange("p (g t) -> p g t", t=T)
                gsl = slice(glo, ghi)
                nc.vector.tensor_scalar(
                    out=gt[:, fs], in0=pot[:, fs], scalar1=TH, scalar2=None,
                    op0=Op.is_gt)
                nc.vector.tensor_tensor(
                    out=thr[:, fs], in0=pot[:, fs], in1=gt[:, fs], op=Op.mult)
                cnt = packed[:, glo:ghi]
                nc.vector.reduce_sum(out=cnt, in_=g3, axis=Ax.X)
                # first spike time: min(32 - cnt, 31)
                nc.vector.tensor_scalar(
                    out=first[:, gsl], in0=cnt, scalar1=32.0, scalar2=-1.0,
                    op0=Op.subtract, op1=Op.mult)
                nc.vector.tensor_scalar(
                    out=first[:, gsl], in0=first[:, gsl], scalar1=31.0,
                    scalar2=None, op0=Op.min)
                # vals_at_first = sum_t thr * (iota_t == first)
                nc.vector.tensor_tensor(
                    out=s3,
                    in0=iota_sb[:, None, :].to_broadcast([128, gn, T]),
                    in1=first[:, gsl, None].to_broadcast([128, gn, T]),
                    op=Op.is_equal)
                nc.vector.tensor_tensor(out=s23, in0=s3, in1=t3, op=Op.mult)
                vals = packed[:, 32 + glo:32 + ghi]
                nc.vector.reduce_sum(out=vals, in_=s23, axis=Ax.X)
                # rowmax = vals * (cnt > 0)
                nc.vector.tensor_scalar(
                    out=has[:, gsl], in0=cnt, scalar1=0.0, scalar2=None,
                    op0=Op.is_gt)
                nc.vector.tensor_tensor(
                    out=packed[:, 64 + glo:64 + ghi], in0=vals, in1=has[:, gsl],
                    op=Op.mult)

            for g in range(G):
                xt = io.tile([128, RS * CH * T], F32, tag="x")
                wt = io.tile([128, RS * CH * K], F32, tag="w")
                nc.sync.dma_start(out=xt[:], in_=x[g, :, :])
                nc.sync.dma_start(out=wt[:], in_=w[g, :, :])
                ps = psp.tile([128, T], F32)
                for c in range(CH):
                    for rs in range(RS):
                        off = rs * 512 + c * 32
                        nc.tensor.matmul(
                            out=ps[rs * 32:(rs + 1) * 32, :],
                            lhsT=wt[:, off:off + K],
                            rhs=xt[:, off:off + T],
                            start=(c == 0),
                            stop=(c == CH - 1),
                            tile_position=(0, rs * 32),
                        )
                nc.scalar.copy(out=pot[:, g * T:(g + 1) * T], in_=ps[:])
                if g == G // 2 - 1:
                    stage_a(0, G // 2)
            stage_a(G // 2, G)

            # 32x32 block transpose: -> [p=(rs,g), free=k] per 32-block
            tp = wk.tile([128, 96], F32)
            nc.vector.transpose(out=tp[:], in_=packed[:])
            cntT = tp[:, 0:32]
            valsT = tp[:, 32:64]
            rowmaxT = tp[:, 64:96]

            # per-branch v = 32 * max_k rowmax;  total = cnt * (vals + v)
            vmax = wk.tile([128, 1], F32)
            nc.vector.reduce_max(out=vmax[:], in_=rowmaxT, axis=Ax.X)
            v32 = wk.tile([128, 1], F32)
            nc.vector.tensor_scalar(
                out=v32[:], in0=vmax[:], scalar1=32.0, scalar2=None, op0=Op.mult)
            tot = wk.tile([128, K], F32)
            nc.vector.tensor_scalar(
                out=tot[:], in0=valsT, scalar1=v32[:], scalar2=None, op0=Op.add)
            tot2 = wk.tile([128, K], F32)
            nc.vector.tensor_tensor(out=tot2[:], in0=tot[:], in1=cntT, op=Op.mult)

            # top-4 with stable (lower index first) tie-break:
            # m4 = 4th largest; keep (tot > m4) plus the first (4 - #gt) ties.
            m8 = wk.tile([128, 8], F32)
            nc.vector.max(out=m8[:], in_=tot2[:])
            sg = wk.tile([128, K], F32)
            eq = wk.tile([128, K], F32)
            nc.vector.tensor_scalar(
                out=sg[:], in0=tot2[:], scalar1=m8[:, 3:4], scalar2=None, op0=Op.is_gt)
            nc.vector.tensor_scalar(
                out=eq[:], in0=tot2[:], scalar1=m8[:, 3:4], scalar2=None,
                op0=Op.is_equal)
            ng = wk.tile([128, 1], F32)
            nc.vector.reduce_sum(out=ng[:], in_=sg[:], axis=Ax.X)
            need = wk.tile([128, 1], F32)
            nc.vector.tensor_scalar(
                out=need[:], in0=ng[:], scalar1=4.0, scalar2=-1.0,
                op0=Op.subtract, op1=Op.mult)
            incl = wk.tile([128, K], F32)
            nc.vector.tensor_tensor_scan(
                out=incl[:], data0=eq[:], data1=zeros[:], initial=0.0,
                op0=Op.add, op1=Op.add)
            eqrank = wk.tile([128, K], F32)
            nc.vector.tensor_tensor(
                out=eqrank[:], in0=incl[:], in1=eq[:], op=Op.subtract)
            seleq = wk.tile([128, K], F32)
            nc.vector.tensor_scalar(
                out=seleq[:], in0=eqrank[:], scalar1=need[:], scalar2=None,
                op0=Op.is_lt)
            eqs = wk.tile([128, K], F32)
            nc.vector.tensor_tensor(out=eqs[:], in0=eq[:], in1=seleq[:], op=Op.mult)
            msum = wk.tile([128, K], F32)
            nc.vector.tensor_tensor(out=msum[:], in0=sg[:], in1=eqs[:], op=Op.add)
            gt0 = wk.tile([128, K], F32)
            nc.vector.tensor_scalar(
                out=gt0[:], in0=tot2[:], scalar1=0.0, scalar2=None, op0=Op.is_gt)
            maskT = wk.tile([128, K], F32)
            nc.vector.tensor_tensor(out=maskT[:], in0=msum[:], in1=gt0[:], op=Op.mult)

            # transpose mask back to [p=(rs,k), free=g] and apply to spikes
            maskA = wk.tile([128, K], F32)
            nc.vector.transpose(out=maskA[:], in_=maskT[:])
            outt = wk.tile([128, G * T], F32)
            for glo, ghi in ((0, G // 2), (G // 2, G)):
                gn = ghi - glo
                fs = slice(glo * T, ghi * T)
                o3 = outt[:, fs].rearrange("p (g t) -> p g t", t=T)
                g3 = gt[:, fs].rearrange("p (g t) -> p g t", t=T)
                nc.vector.tensor_tensor(
                    out=o3, in0=g3,
                    in1=maskA[:, glo:ghi, None].to_broadcast([128, gn, T]),
                    op=Op.mult)
                nc.sync.dma_start(out=out[:, fs], in_=outt[:, fs])

    nc.compile()
    return nc


def prep_inputs(rec_field, W):
    """Host-side relayout into the per-core DMA-friendly layouts."""
    rec_field = np.asarray(rec_field, dtype=np.float32)
    W = np.asarray(W, dtype=np.float32)
    xr = rec_field[:, 0].transpose(1, 2, 0)            # (RF, L, T)
    x6 = xr.reshape(NCORES, G, RS, CH, 128, T)         # (d, g, rs, c, p, t)
    xh = np.ascontiguousarray(x6.transpose(0, 1, 4, 2, 3, 5)).reshape(
        NCORES, G, 128, RS * CH * T)
    wr = W[:, :, 0, :].transpose(0, 2, 1)              # (RF, L, K)
    w6 = wr.reshape(NCORES, G, RS, CH, 128, K)
    wh = np.ascontiguousarray(w6.transpose(0, 1, 4, 2, 3, 5)).reshape(
        NCORES, G, 128, RS * CH * K)
    return xh, wh


def make_in_maps(rec_field, W):
    xh, wh = prep_inputs(rec_field, W)
    iota = np.ascontiguousarray(
        np.tile(np.arange(T, dtype=np.float32), (128, 1)))
    return [{"x": xh[d], "w": wh[d], "iota_t": iota} for d in range(NCORES)]


def assemble_output(results):
    """results: per-core dicts with 'out' (128, 512) -> full (T,1,K,RF)."""
    out_full = np.zeros((T, 1, K, RF), np.float32)
    for d in range(NCORES):
        o = np.asarray(results[d]["out"]).reshape(RS, K, G, T)
        o = o.transpose(3, 1, 2, 0).reshape(T, K, G * RS)   # (t, k, b=g*4+rs)
        out_full[:, 0, :, d * (G * RS):(d + 1) * (G * RS)] = o
    return out_full


def get_nc():
    if "nc" not in _CACHE:
        _CACHE["nc"] = build()
    return _CACHE["nc"]


def kernel(rec_field, W, reward=None, **_unused):
    nc = get_nc()
    in_maps = make_in_maps(rec_field, W)
    res = bass_utils.run_bass_kernel_spmd(nc, in_maps, core_ids=list(range(NCORES)))
    return assemble_output(res.results)
